# revision 11
# baseline (speedup 1.0000x reference)
"""Trainium2 Bass kernel for causal self-attention (B=4, T=2048, C=2048, H=16).

Sharding: 8 cores = 4 batches x 2 head-groups (8 heads each).

v3 design: hybrid fp8-e4m3 DoubleRow + fp16.
  - QKV projections run in fp8 DoubleRow (2x PE throughput), EXCEPT the
    first 256 tokens, which are accumulated from fp16 operands into the
    same PSUM banks. Early query rows attend over few keys, so softmax
    averaging cannot suppress fp8 quantization noise there; rows >= 256
    average over >= 256 keys and fp8 noise washes out.
  - Scores/PV (fp16 operands, fp32 PSUM) and the output projection
    (fp16) are unchanged from v2.
  - All intermediates SBUF-resident; B (QK proj + RoPE) emitted
    head-pair-interleaved with C (SDPA); reciprocal_approx_fast for the
    softmax denominator; V/proj biases folded into host postprocess.
  - Diag masks and RoPE swap-half muls go to the idle GpSimd engine.

Per core (its batch b, its 8 heads):
  A) v = x @ Wv            -> v_sb [128, 16, 1024] f16 resident
  B) per feature f (q_h/k_h alternating): qkT chunk + bias + RoPE ->
     qk ring slot [128, 2048] f16
  C) per head: k-tile scores, exp (ACT), p_sum adds (DVE f16 2x),
     PV psum accumulation, ones-matmul denominator,
     reciprocal_approx_fast, normalize -> y[h] f16
  D) partial_out = y^T @ wp_rows -> [T, C] f32 partial
Host sums core pairs per batch and adds b_proj + bv @ w_proj.
"""

import sys

import numpy as np

sys.path.insert(0, "/opt/trn_rl_repo")

import ml_dtypes  # noqa: E402

import concourse.bass as bass  # noqa: E402,F401
import concourse.mybir as mybir  # noqa: E402
import concourse.tile as tile  # noqa: E402
from concourse import bacc  # noqa: E402

F32 = mybir.dt.float32
F16 = mybir.dt.float16
F8 = mybir.dt.float8e4
DR = mybir.MatmulPerfMode.DoubleRow
AF = mybir.ActivationFunctionType
NP8 = ml_dtypes.float8_e4m3fn

B, T, C = 4, 2048, 2048
H, D = 16, 128
HPC = 8            # heads per core
P = 128
NT = 512           # matmul moving free dim
TT = T // NT       # 4 token tiles
CC = C // P        # 16 contraction chunks over C
NF = 2 * HPC       # 16 features (q_h / k_h interleaved)
NSLOT = 5          # qk ring slots
HD2 = D // 2       # rope half
PATCH = 256        # first tokens computed in fp16
ROPE_BASE = 10000.0

_CACHE = {}


def build_program():
    nc = bacc.Bacc(name="csa_v3")

    # fp8 operands (DoubleRow layouts, [p, c, n] chunked)
    xt8 = nc.dram_tensor("xt8", (TT * P, CC * NT), F8, kind="ExternalInput")
    wv8 = nc.dram_tensor("wv8", (P, CC * HPC * D), F8, kind="ExternalInput")
    wqk8 = nc.dram_tensor("wqk8", (NF * P, C), F8, kind="ExternalInput")
    # fp16 patch operands for tokens [0, PATCH)
    xt16p = nc.dram_tensor("xt16p", (P, CC * PATCH), F16,
                           kind="ExternalInput")
    wv16 = nc.dram_tensor("wv16", (C, HPC * D), F16, kind="ExternalInput")
    wqk16 = nc.dram_tensor("wqk16", (NF * P, C), F16, kind="ExternalInput")

    bqk = nc.dram_tensor("bqk", (P, NF), F32, kind="ExternalInput")
    cs = nc.dram_tensor("cs", (P, T), F16, kind="ExternalInput")
    sw = nc.dram_tensor("sw", (P, T), F16, kind="ExternalInput")
    tri = nc.dram_tensor("tri", (P, P), F16, kind="ExternalInput")
    onesm = nc.dram_tensor("onesm", (P, P), F16, kind="ExternalInput")
    wp = nc.dram_tensor("wp", (HPC * D, C), F16, kind="ExternalInput")
    out = nc.dram_tensor("out", (T, C), F32, kind="ExternalOutput")

    def mm(o, lhsT, rhs, **kw):
        nc.tensor.matmul(o, lhsT, rhs, **kw)

    with tile.TileContext(nc) as tc, nc.allow_low_precision(
        "fp16 softmax partial sums; verified on host against fp32 reference"
    ):
        with (
            tc.tile_pool(name="const", bufs=1) as const_p,
            tc.tile_pool(name="vsb", bufs=1) as v_p,
            tc.tile_pool(name="qk", bufs=1) as qk_p,
            tc.tile_pool(name="ybuf", bufs=1) as y_p,
            tc.tile_pool(name="work", bufs=1) as wk,
        ):
            v_sb = v_p.tile([P, T // P, HPC * D], F16, tag="vsb", name="vsb")
            y_t = [
                y_p.tile([P, T], F16, tag=f"y{h}", name=f"y{h}")
                for h in range(HPC)
            ]

            with tc.tile_pool(name="xt_res", bufs=1) as xt_p:
                with (
                    tc.tile_pool(name="wv_p", bufs=1) as wv_p,
                    tc.tile_pool(name="psA", bufs=1, space="PSUM") as psA,
                ):
                    # ---- first wave of loads, spread across engines ----
                    xp = xt_p.tile([P, CC, PATCH], F16, tag="xt16p",
                                   name="xt16p")
                    nc.sync.dma_start(xp[:], xt16p[:])
                    wv16_t = [None] * CC
                    for c in range(CC):
                        w_ = wv_p.tile([P, NT], F16, tag=f"wv16_{c}",
                                       name=f"wv16_{c}")
                        eng = (nc.gpsimd, nc.scalar, nc.sync)[c % 3]
                        eng.dma_start(w_[:], wv16[c * P:(c + 1) * P, 0:NT])
                        wv16_t[c] = w_
                    wv8_t = wv_p.tile([P, CC, HPC * D], F8, tag="wv8",
                                      name="wv8")
                    nc.gpsimd.dma_start(wv8_t[:], wv8[:])
                    xt8_t = [None] * TT
                    for t in range(TT):
                        x_ = xt_p.tile([P, CC, NT], F8, tag=f"xt8_{t}",
                                       name=f"xt8_{t}")
                        nc.scalar.dma_start(
                            x_[:], xt8[t * P:(t + 1) * P, :])
                        xt8_t[t] = x_

                    # ---------------- phase A: V ----------------
                    # fp16 patch: token blocks m=0,1 (n-outer waves)
                    for n in range(2):
                        if n == 1:
                            for c in range(CC):
                                w_ = wv_p.tile([P, NT], F16,
                                               tag=f"wv16_{c}",
                                               name=f"wv16_{c}")
                                eng = (nc.gpsimd, nc.scalar, nc.sync)[c % 3]
                                eng.dma_start(
                                    w_[:],
                                    wv16[c * P:(c + 1) * P, NT:2 * NT])
                                wv16_t[c] = w_
                        for m in range(2):
                            ps = psA.tile([P, NT], F32, tag="psa", bufs=4,
                                          name="psa")
                            for c in range(CC):
                                mm(ps[:], xp[:, c, m * P:(m + 1) * P],
                                   wv16_t[c][:],
                                   start=(c == 0), stop=(c == CC - 1))
                            nc.scalar.activation(
                                v_sb[:, m, n * NT:(n + 1) * NT], ps[:],
                                AF.Identity, scale=1.0 / 64.0)
                    # fp8 DoubleRow: token blocks m=2..15
                    for mtok in range(2, 16):
                        t, m = mtok // 4, mtok % 4
                        psd = [
                            psA.tile([P, NT], F32, tag="psa", bufs=4,
                                     name="psa")
                            for _ in range(2)
                        ]
                        for cp in range(CC // 2):
                            lhsT = xt8_t[t][:, 2 * cp:2 * cp + 2,
                                            m * P:(m + 1) * P]
                            for n in range(2):
                                mm(psd[n][:], lhsT,
                                   wv8_t[:, 2 * cp:2 * cp + 2,
                                         n * NT:(n + 1) * NT],
                                   start=(cp == 0), stop=(cp == 7),
                                   perf_mode=DR)
                        for n in range(2):
                            nc.scalar.activation(
                                v_sb[:, mtok, n * NT:(n + 1) * NT],
                                psd[n][:], AF.Identity, scale=1.0 / 64.0)

                # consts (needed from B on)
                bqk_t = const_p.tile([P, NF], F32, tag="bqk", name="bqk")
                nc.sync.dma_start(bqk_t[:], bqk[:])
                cs_t = const_p.tile([P, T], F16, tag="cs", name="cs")
                nc.sync.dma_start(cs_t[:], cs[:])
                sw_t = const_p.tile([P, T], F16, tag="sw", name="sw")
                nc.sync.dma_start(sw_t[:], sw[:])
                tri_t = const_p.tile([P, P], F16, tag="tri", name="tri")
                nc.sync.dma_start(tri_t[:], tri[:])
                ones_t = const_p.tile([P, P], F16, tag="ones", name="ones")
                nc.sync.dma_start(ones_t[:], onesm[:])

                # ---------- phases B+C interleaved per head ----------
                with (
                    tc.tile_pool(name="wqk_p", bufs=1) as wqk_p,
                    tc.tile_pool(name="psBC", bufs=1, space="PSUM") as psBC,
                ):
                    slots = [None] * NSLOT

                    def emit_B(f):
                        wqf8 = wqk_p.tile([P, CC, P], F8, tag="wq8", bufs=2,
                                          name="wq8")
                        nc.gpsimd.dma_start(wqf8[:],
                                            wqk8[f * P:(f + 1) * P, :])
                        wqf16 = wqk_p.tile([P, CC, P], F16, tag="wq16",
                                           bufs=2, name="wq16")
                        nc.gpsimd.dma_start(wqf16[:],
                                            wqk16[f * P:(f + 1) * P, :])
                        psB = [
                            psBC.tile([P, NT], F32, tag="psB", bufs=4,
                                      name="psB")
                            for _ in range(TT)
                        ]
                        # t=0 cols [0, PATCH): fp16
                        for c in range(CC):
                            mm(psB[0][:, 0:PATCH], wqf16[:, c, :],
                               xp[:, c, :],
                               start=(c == 0), stop=(c == CC - 1))
                        # t=0 cols [PATCH, NT): fp8 DR
                        for cp in range(CC // 2):
                            mm(psB[0][:, PATCH:NT],
                               wqf8[:, 2 * cp:2 * cp + 2, :],
                               xt8_t[0][:, 2 * cp:2 * cp + 2, PATCH:NT],
                               start=(cp == 0), stop=(cp == 7),
                               perf_mode=DR)
                        # t>=1: fp8 DR
                        for cp in range(CC // 2):
                            for t in range(1, TT):
                                mm(psB[t][:],
                                   wqf8[:, 2 * cp:2 * cp + 2, :],
                                   xt8_t[t][:, 2 * cp:2 * cp + 2, :],
                                   start=(cp == 0), stop=(cp == 7),
                                   perf_mode=DR)
                        slot = qk_p.tile([P, T], F16, tag=f"qks{f % NSLOT}",
                                         name=f"qks{f % NSLOT}")
                        slots[f % NSLOT] = slot
                        for t in range(TT):
                            sl = slice(t * NT, (t + 1) * NT)
                            raw = wk.tile([P, NT], F16, tag="raw", bufs=3,
                                          name="raw")
                            nc.scalar.activation(
                                raw[:], psB[t][:], AF.Identity,
                                bias=bqk_t[:, f:f + 1], scale=1.0 / 64.0)
                            rsw = wk.tile([P, NT], F16, tag="rsw", bufs=3,
                                          name="rsw")
                            nc.scalar.activation(
                                rsw[0:HD2, :], psB[t][HD2:P, :], AF.Identity,
                                bias=bqk_t[HD2:P, f:f + 1], scale=1.0 / 64.0)
                            nc.scalar.activation(
                                rsw[HD2:P, :], psB[t][0:HD2, :], AF.Identity,
                                bias=bqk_t[0:HD2, f:f + 1], scale=1.0 / 64.0)
                            nc.vector.tensor_mul(slot[:, sl], raw[:],
                                                 cs_t[:, sl])
                            tmp = wk.tile([P, NT], F16, tag="tmpb", bufs=2,
                                          name="tmpb")
                            nc.gpsimd.tensor_mul(tmp[:], rsw[:], sw_t[:, sl])
                            nc.vector.tensor_add(slot[:, sl], slot[:, sl],
                                                 tmp[:])

                    def emit_C(h):
                        qh = slots[(2 * h) % NSLOT]
                        kh = slots[(2 * h + 1) % NSLOT]
                        for t in range(TT):
                            psy = psBC.tile([P, NT], F32, tag="psy", bufs=1,
                                            name="psy")
                            p_sum = wk.tile([P, NT], F16, tag="ps_s", bufs=2,
                                            name="ps_s")
                            njt = 4 * t + 4
                            for j in range(njt):
                                diag = j >= 4 * t
                                off = (j - 4 * t) * P if diag else 0
                                pss = psBC.tile([P, NT], F32, tag="pss",
                                                bufs=2, name="pss")
                                mm(pss[:, off:], kh[:, j * P:(j + 1) * P],
                                   qh[:, t * NT + off:(t + 1) * NT],
                                   start=True, stop=True)
                                if j == 0:
                                    tgt = p_sum
                                else:
                                    tgt = wk.tile([P, NT], F16, tag="p",
                                                  bufs=3, name="p")
                                nc.scalar.activation(tgt[:, off:],
                                                     pss[:, off:], AF.Exp)
                                if diag:
                                    nc.gpsimd.tensor_mul(
                                        tgt[:, off:off + P],
                                        tgt[:, off:off + P], tri_t[:])
                                if j > 0:
                                    nc.vector.tensor_add(
                                        p_sum[:, off:], p_sum[:, off:],
                                        tgt[:, off:])
                                mm(psy[:, off:],
                                   v_sb[:, j, h * P:(h + 1) * P],
                                   tgt[:, off:],
                                   start=(j == 0), stop=(j == njt - 1))
                            den = psBC.tile([P, NT], F32, tag="den", bufs=1,
                                            name="den")
                            mm(den[:], ones_t[:], p_sum[:],
                               start=True, stop=True)
                            rec = wk.tile([P, NT], F32, tag="rec", bufs=1,
                                          name="rec")
                            nc.vector.reciprocal_approx_fast(rec[:], den[:])
                            nc.vector.tensor_mul(
                                y_t[h][:, t * NT:(t + 1) * NT], psy[:],
                                rec[:])

                    for h in range(HPC):
                        emit_B(2 * h)
                        emit_B(2 * h + 1)
                        emit_C(h)

            # ---------------- phase D: projection ----------------
            with (
                tc.tile_pool(name="wp_p", bufs=1) as wp_p,
                tc.tile_pool(name="ob_p", bufs=1) as ob_p,
                tc.tile_pool(name="psD", bufs=1, space="PSUM") as psD_p,
            ):
                wp_t = []
                for hh in range(HPC):
                    w_ = wp_p.tile([P, C], F16, tag=f"wp{hh}", name=f"wp{hh}")
                    nc.gpsimd.dma_start(w_[:], wp[hh * P:(hh + 1) * P, :])
                    wp_t.append(w_)
                for m in range(T // P):
                    msl = slice(m * P, (m + 1) * P)
                    psD = [
                        psD_p.tile([P, NT], F32, tag="psD", bufs=8,
                                   name="psD")
                        for _ in range(4)
                    ]
                    for hh in range(HPC):
                        lhsT = y_t[hh][:, msl]
                        for oc in range(4):
                            mm(psD[oc][:], lhsT,
                               wp_t[hh][:, oc * NT:(oc + 1) * NT],
                               start=(hh == 0), stop=(hh == HPC - 1))
                    ob = ob_p.tile([P, C], F32, tag="ob", bufs=3, name="ob")
                    for oc in range(4):
                        dst = ob[:, oc * NT:(oc + 1) * NT]
                        if oc % 2 == 0:
                            nc.scalar.copy(dst, psD[oc][:])
                        else:
                            nc.vector.tensor_copy(dst, psD[oc][:])
                    nc.sync.dma_start(out[msl, :], ob[:])

    nc.finalize()
    return nc


def _chunked(a, inner):
    """[CC*P, inner_cols] -> [P, CC*inner_cols] with (p, c, n) layout."""
    return np.ascontiguousarray(
        a.reshape(CC, P, inner).transpose(1, 0, 2).reshape(P, CC * inner))


def prep_inputs(x, w_attn, b_attn, w_proj, b_proj):
    """Build the 8 per-core input maps from full inputs."""
    x = np.asarray(x, dtype=np.float32)
    w_attn = np.asarray(w_attn, dtype=np.float32)
    b_attn = np.asarray(b_attn, dtype=np.float32)
    w_proj = np.asarray(w_proj, dtype=np.float32)

    scale = np.float32(1.0 / np.sqrt(D))

    inv_freq = 1.0 / (ROPE_BASE ** (np.arange(0, D, 2, dtype=np.float32) / D))
    tpos = np.arange(T, dtype=np.float32)
    ang = np.outer(tpos, inv_freq)  # [T, 64]
    cos_t, sin_t = np.cos(ang).T, np.sin(ang).T  # [64, T]
    cs = np.ascontiguousarray(
        np.concatenate([cos_t, cos_t], axis=0).astype(np.float16))
    sw = np.ascontiguousarray(
        np.concatenate([-sin_t, sin_t], axis=0).astype(np.float16))

    qq = np.arange(P)
    kk = np.arange(P)[:, None]
    tri = np.ascontiguousarray(
        (qq[None, :] >= kk).astype(np.float16))  # [128,128] causal triangle
    onesm = np.ones((P, P), dtype=np.float16)

    in_maps = []
    for core in range(8):
        b = core // 2
        hg = core % 2
        heads = list(range(hg * HPC, (hg + 1) * HPC))

        # interleaved feature order: q_0, k_0, q_1, k_1, ...
        wqk_rows = []
        bqk_cols = []
        for h in heads:
            qc = np.arange(h * D, (h + 1) * D)
            kc = qc + C
            for cols, s in ((qc, scale), (kc, np.float32(1.0))):
                wsel = w_attn[:, cols] * s  # [2048, 128]
                wqk_rows.append(_chunked(wsel, P))  # [128, 2048]
                bqk_cols.append((b_attn[cols] * s).astype(np.float32))
        # weights std ~0.022 sits in e4m3's subnormal range; pre-scale by
        # 64 (descaled via ACT scale=1/64 on the PSUM read) so fp8 gets a
        # full mantissa. Patch fp16 weights get the same scale so the
        # shared PSUM banks stay uniform.
        wqk_f32 = np.concatenate(wqk_rows, axis=0) * 64.0  # [16*128, 2048]
        wqk8_s = np.ascontiguousarray(wqk_f32.astype(NP8))
        wqk16_s = np.ascontiguousarray(wqk_f32.astype(np.float16))
        bqk_s = np.ascontiguousarray(np.stack(bqk_cols, axis=1))  # [128, 16]

        qcols = np.concatenate(
            [np.arange(h * D, (h + 1) * D) for h in heads])
        vcols = qcols + 2 * C
        wv_f32 = w_attn[:, vcols] * 64.0  # [2048, 1024]
        wv16_s = np.ascontiguousarray(wv_f32.astype(np.float16))
        wv8_s = _chunked(wv_f32, HPC * D).astype(NP8)  # [128, 16*1024]
        wp_s = np.ascontiguousarray(w_proj[qcols, :].astype(np.float16))

        xT = x[b].T  # [C, T]
        xt16p_s = _chunked(xT[:, 0:PATCH], PATCH).astype(np.float16)
        # xt8: per t, [128, 16*512] chunked; stacked rows [4*128, 16*512]
        xt8_s = np.concatenate(
            [_chunked(xT[:, t * NT:(t + 1) * NT], NT) for t in range(TT)],
            axis=0).astype(NP8)

        in_maps.append({
            "xt8": np.ascontiguousarray(xt8_s),
            "wv8": np.ascontiguousarray(wv8_s),
            "wqk8": wqk8_s,
            "xt16p": np.ascontiguousarray(xt16p_s),
            "wv16": wv16_s,
            "wqk16": wqk16_s,
            "bqk": bqk_s,
            "cs": cs, "sw": sw, "tri": tri, "onesm": onesm, "wp": wp_s,
        })
    return in_maps


def _get_program():
    if "nc" not in _CACHE:
        _CACHE["nc"] = build_program()
    return _CACHE["nc"]


def _postprocess(outs, b_attn, w_proj, b_proj):
    b_attn = np.asarray(b_attn, dtype=np.float32)
    w_proj = np.asarray(w_proj, dtype=np.float32)
    b_proj = np.asarray(b_proj, dtype=np.float32)
    # v-bias and proj-bias are linear terms folded in on the host
    bias_full = b_attn[2 * C:3 * C] @ w_proj + b_proj  # [C]
    return np.stack(
        [outs[2 * b] + outs[2 * b + 1] + bias_full[None, :]
         for b in range(B)]
    ).astype(np.float32)


def _run(inputs, trace=False):
    from concourse.bass_utils import run_bass_kernel_spmd

    nc = _get_program()
    in_maps = prep_inputs(
        inputs["x"], inputs["w_attn"], inputs["b_attn"],
        inputs["w_proj"], inputs["b_proj"],
    )
    res = run_bass_kernel_spmd(nc, in_maps, core_ids=list(range(8)),
                               trace=trace)
    full = _postprocess([r["out"] for r in res.results],
                        inputs["b_attn"], inputs["w_proj"],
                        inputs["b_proj"])
    return full, res


def kernel(**inputs):
    full, _ = _run(inputs, trace=False)
    return full


if __name__ == "__main__":
    _get_program()
    print("built ok")


# revision 14
# speedup vs baseline: 1.0732x; 1.0732x over previous
"""Trainium2 Bass kernel for causal self-attention (B=4, T=2048, C=2048, H=16).

Sharding: 8 cores = 4 batches x 2 head-groups (8 heads each).

v3 design: hybrid fp8-e4m3 DoubleRow + fp16.
  - QKV projections run in fp8 DoubleRow (2x PE throughput), EXCEPT the
    first 256 tokens, which are accumulated from fp16 operands into the
    same PSUM banks. Early query rows attend over few keys, so softmax
    averaging cannot suppress fp8 quantization noise there; rows >= 256
    average over >= 256 keys and fp8 noise washes out.
  - Scores/PV (fp16 operands, fp32 PSUM) and the output projection
    (fp16) are unchanged from v2.
  - All intermediates SBUF-resident; B (QK proj + RoPE) emitted
    head-pair-interleaved with C (SDPA); reciprocal_approx_fast for the
    softmax denominator; V/proj biases folded into host postprocess.
  - Diag masks and RoPE swap-half muls go to the idle GpSimd engine.

Per core (its batch b, its 8 heads):
  A) v = x @ Wv            -> v_sb [128, 16, 1024] f16 resident
  B) per feature f (q_h/k_h alternating): qkT chunk + bias + RoPE ->
     qk ring slot [128, 2048] f16
  C) per head: k-tile scores, exp (ACT), p_sum adds (DVE f16 2x),
     PV psum accumulation, ones-matmul denominator,
     reciprocal_approx_fast, normalize -> y[h] f16
  D) partial_out = y^T @ wp_rows -> [T, C] f32 partial
Host sums core pairs per batch and adds b_proj + bv @ w_proj.
"""

import sys

import numpy as np

sys.path.insert(0, "/opt/trn_rl_repo")

import ml_dtypes  # noqa: E402

import concourse.bass as bass  # noqa: E402,F401
import concourse.mybir as mybir  # noqa: E402
import concourse.tile as tile  # noqa: E402
from concourse import bacc  # noqa: E402

F32 = mybir.dt.float32
F16 = mybir.dt.float16
F8 = mybir.dt.float8e4
DR = mybir.MatmulPerfMode.DoubleRow
AF = mybir.ActivationFunctionType
NP8 = ml_dtypes.float8_e4m3fn

B, T, C = 4, 2048, 2048
H, D = 16, 128
HPC = 8            # heads per core
P = 128
NT = 512           # matmul moving free dim
TT = T // NT       # 4 token tiles
CC = C // P        # 16 contraction chunks over C
NF = 2 * HPC       # 16 features (q_h / k_h interleaved)
NSLOT = 5          # qk ring slots
HD2 = D // 2       # rope half
PATCH = 256        # first tokens computed in fp16
ROPE_BASE = 10000.0

_CACHE = {}


def build_program():
    nc = bacc.Bacc(name="csa_v3")

    # fp8 operands (DoubleRow layouts, [p, c, n] chunked)
    xt8 = nc.dram_tensor("xt8", (TT * P, CC * NT), F8, kind="ExternalInput")
    wv8 = nc.dram_tensor("wv8", (P, CC * HPC * D), F8, kind="ExternalInput")
    wqk8 = nc.dram_tensor("wqk8", (NF * P, C), F8, kind="ExternalInput")
    # fp16 patch operands for tokens [0, PATCH)
    xt16p = nc.dram_tensor("xt16p", (P, CC * PATCH), F16,
                           kind="ExternalInput")
    wv16 = nc.dram_tensor("wv16", (C, HPC * D), F16, kind="ExternalInput")
    wqk16 = nc.dram_tensor("wqk16", (NF * P, C), F16, kind="ExternalInput")

    bqk = nc.dram_tensor("bqk", (P, NF), F32, kind="ExternalInput")
    cs = nc.dram_tensor("cs", (P, T), F16, kind="ExternalInput")
    sw = nc.dram_tensor("sw", (P, T), F16, kind="ExternalInput")
    tri = nc.dram_tensor("tri", (P, P), F16, kind="ExternalInput")
    onesm = nc.dram_tensor("onesm", (P, P), F16, kind="ExternalInput")
    wp = nc.dram_tensor("wp", (HPC * D, C), F16, kind="ExternalInput")
    out = nc.dram_tensor("out", (T, C), F32, kind="ExternalOutput")

    def mm(o, lhsT, rhs, **kw):
        nc.tensor.matmul(o, lhsT, rhs, **kw)

    with tile.TileContext(nc) as tc, nc.allow_low_precision(
        "fp16 softmax partial sums; verified on host against fp32 reference"
    ):
        with (
            tc.tile_pool(name="const", bufs=1) as const_p,
            tc.tile_pool(name="vsb", bufs=1) as v_p,
            tc.tile_pool(name="qk", bufs=1) as qk_p,
            tc.tile_pool(name="ybuf", bufs=1) as y_p,
            tc.tile_pool(name="work", bufs=1) as wk,
        ):
            v_sb = v_p.tile([P, T // P, HPC * D], F16, tag="vsb", name="vsb")
            y_t = [
                y_p.tile([P, T], F16, tag=f"y{h}", name=f"y{h}")
                for h in range(HPC)
            ]

            with tc.tile_pool(name="xt_res", bufs=1) as xt_p:
                with (
                    tc.tile_pool(name="wv_p", bufs=1) as wv_p,
                    tc.tile_pool(name="psA", bufs=1, space="PSUM") as psA,
                ):
                    # ---- first wave: fp8 operands (2+4 big DMAs) ----
                    wv8_t = wv_p.tile([P, CC, HPC * D], F8, tag="wv8",
                                      name="wv8")
                    nc.gpsimd.dma_start(wv8_t[:], wv8[:])
                    xt8_t = [None] * TT
                    for t in range(TT):
                        x_ = xt_p.tile([P, CC, NT], F8, tag=f"xt8_{t}",
                                       name=f"xt8_{t}")
                        eng = (nc.scalar, nc.sync)[t % 2]
                        eng.dma_start(
                            x_[:], xt8[t * P:(t + 1) * P, :])
                        xt8_t[t] = x_
                    xp = xt_p.tile([P, CC, PATCH], F16, tag="xt16p",
                                   name="xt16p")
                    nc.sync.dma_start(xp[:], xt16p[:])
                    wv16_t = [None] * CC
                    for c in range(CC):
                        w_ = wv_p.tile([P, NT], F16, tag=f"wv16_{c}",
                                       name=f"wv16_{c}")
                        eng = (nc.gpsimd, nc.scalar, nc.sync)[c % 3]
                        eng.dma_start(w_[:], wv16[c * P:(c + 1) * P, 0:NT])
                        wv16_t[c] = w_

                    # ---------------- phase A: V ----------------
                    # fp8 DoubleRow: token blocks m=2..15
                    for mtok in range(2, 16):
                        t, m = mtok // 4, mtok % 4
                        psd = [
                            psA.tile([P, NT], F32, tag="psa", bufs=4,
                                     name="psa")
                            for _ in range(2)
                        ]
                        for cp in range(CC // 2):
                            lhsT = xt8_t[t][:, 2 * cp:2 * cp + 2,
                                            m * P:(m + 1) * P]
                            for n in range(2):
                                mm(psd[n][:], lhsT,
                                   wv8_t[:, 2 * cp:2 * cp + 2,
                                         n * NT:(n + 1) * NT],
                                   start=(cp == 0), stop=(cp == 7),
                                   perf_mode=DR)
                        for n in range(2):
                            nc.scalar.activation(
                                v_sb[:, mtok, n * NT:(n + 1) * NT],
                                psd[n][:], AF.Identity, scale=1.0 / 64.0)
                    # fp16 patch: token blocks m=0,1 (n-outer waves)
                    for n in range(2):
                        if n == 1:
                            for c in range(CC):
                                w_ = wv_p.tile([P, NT], F16,
                                               tag=f"wv16_{c}",
                                               name=f"wv16_{c}")
                                eng = (nc.gpsimd, nc.scalar, nc.sync)[c % 3]
                                eng.dma_start(
                                    w_[:],
                                    wv16[c * P:(c + 1) * P, NT:2 * NT])
                                wv16_t[c] = w_
                        for m in range(2):
                            ps = psA.tile([P, NT], F32, tag="psa", bufs=4,
                                          name="psa")
                            for c in range(CC):
                                mm(ps[:], xp[:, c, m * P:(m + 1) * P],
                                   wv16_t[c][:],
                                   start=(c == 0), stop=(c == CC - 1))
                            nc.scalar.activation(
                                v_sb[:, m, n * NT:(n + 1) * NT], ps[:],
                                AF.Identity, scale=1.0 / 64.0)

                # consts (needed from B on)
                bqk_t = const_p.tile([P, NF], F32, tag="bqk", name="bqk")
                nc.sync.dma_start(bqk_t[:], bqk[:])
                cs_t = const_p.tile([P, T], F16, tag="cs", name="cs")
                nc.sync.dma_start(cs_t[:], cs[:])
                sw_t = const_p.tile([P, T], F16, tag="sw", name="sw")
                nc.sync.dma_start(sw_t[:], sw[:])
                tri_t = const_p.tile([P, P], F16, tag="tri", name="tri")
                nc.sync.dma_start(tri_t[:], tri[:])
                ones_t = const_p.tile([P, P], F16, tag="ones", name="ones")
                nc.sync.dma_start(ones_t[:], onesm[:])

                # ---------- phases B+C interleaved per head ----------
                with (
                    tc.tile_pool(name="wqk_p", bufs=1) as wqk_p,
                    tc.tile_pool(name="psBC", bufs=1, space="PSUM") as psBC,
                ):
                    slots = [None] * NSLOT

                    def emit_B(f):
                        wqf8 = wqk_p.tile([P, CC, P], F8, tag="wq8", bufs=2,
                                          name="wq8")
                        nc.gpsimd.dma_start(wqf8[:],
                                            wqk8[f * P:(f + 1) * P, :])
                        wqf16 = wqk_p.tile([P, CC, P], F16, tag="wq16",
                                           bufs=2, name="wq16")
                        nc.gpsimd.dma_start(wqf16[:],
                                            wqk16[f * P:(f + 1) * P, :])
                        psB = [
                            psBC.tile([P, NT], F32, tag="psB", bufs=4,
                                      name="psB")
                            for _ in range(TT)
                        ]
                        # t=0 cols [0, PATCH): fp16
                        for c in range(CC):
                            mm(psB[0][:, 0:PATCH], wqf16[:, c, :],
                               xp[:, c, :],
                               start=(c == 0), stop=(c == CC - 1))
                        # t=0 cols [PATCH, NT): fp8 DR
                        for cp in range(CC // 2):
                            mm(psB[0][:, PATCH:NT],
                               wqf8[:, 2 * cp:2 * cp + 2, :],
                               xt8_t[0][:, 2 * cp:2 * cp + 2, PATCH:NT],
                               start=(cp == 0), stop=(cp == 7),
                               perf_mode=DR)
                        # t>=1: fp8 DR
                        for cp in range(CC // 2):
                            for t in range(1, TT):
                                mm(psB[t][:],
                                   wqf8[:, 2 * cp:2 * cp + 2, :],
                                   xt8_t[t][:, 2 * cp:2 * cp + 2, :],
                                   start=(cp == 0), stop=(cp == 7),
                                   perf_mode=DR)
                        slot = qk_p.tile([P, T], F16, tag=f"qks{f % NSLOT}",
                                         name=f"qks{f % NSLOT}")
                        slots[f % NSLOT] = slot
                        ADD, MUL = (mybir.AluOpType.add,
                                    mybir.AluOpType.mult)
                        for t in range(TT):
                            sl = slice(t * NT, (t + 1) * NT)
                            # slot = (psB + 64b) * (cs/64)  -- fused on DVE
                            nc.vector.scalar_tensor_tensor(
                                slot[:, sl], psB[t][:], bqk_t[:, f:f + 1],
                                cs_t[:, sl], op0=ADD, op1=MUL)
                            # swap halves: ACT bias-add (64-scaled out),
                            # then Pool multiplies by sw/64
                            rsw = wk.tile([P, NT], F16, tag="rsw", bufs=3,
                                          name="rsw")
                            nc.scalar.activation(
                                rsw[0:HD2, :], psB[t][HD2:P, :], AF.Identity,
                                bias=bqk_t[HD2:P, f:f + 1])
                            nc.scalar.activation(
                                rsw[HD2:P, :], psB[t][0:HD2, :], AF.Identity,
                                bias=bqk_t[0:HD2, f:f + 1])
                            tmp = wk.tile([P, NT], F16, tag="tmpb", bufs=2,
                                          name="tmpb")
                            nc.gpsimd.tensor_mul(tmp[:], rsw[:],
                                                 sw_t[:, sl])
                            nc.vector.tensor_add(slot[:, sl], slot[:, sl],
                                                 tmp[:])

                    def emit_C(h):
                        qh = slots[(2 * h) % NSLOT]
                        kh = slots[(2 * h + 1) % NSLOT]
                        for t in range(TT):
                            psy = psBC.tile([P, NT], F32, tag="psy", bufs=1,
                                            name="psy")
                            ps_e = wk.tile([P, NT], F16, tag="ps_e", bufs=2,
                                           name="ps_e")
                            ps_o = wk.tile([P, NT], F16, tag="ps_o", bufs=2,
                                           name="ps_o")
                            if t == 0:
                                nc.vector.memset(ps_o[:, 0:P], 0.0)
                            njt = 4 * t + 4
                            for j in range(njt):
                                diag = j >= 4 * t
                                off = (j - 4 * t) * P if diag else 0
                                pss = psBC.tile([P, NT], F32, tag="pss",
                                                bufs=2, name="pss")
                                mm(pss[:, off:], kh[:, j * P:(j + 1) * P],
                                   qh[:, t * NT + off:(t + 1) * NT],
                                   start=True, stop=True)
                                if j == 0:
                                    tgt = ps_e
                                elif j == 1:
                                    tgt = ps_o
                                else:
                                    tgt = wk.tile([P, NT], F16, tag="p",
                                                  bufs=3, name="p")
                                nc.scalar.activation(tgt[:, off:],
                                                     pss[:, off:], AF.Exp)
                                if diag:
                                    nc.vector.tensor_mul(
                                        tgt[:, off:off + P],
                                        tgt[:, off:off + P], tri_t[:])
                                if j >= 2:
                                    acc = ps_e if j % 2 == 0 else ps_o
                                    eng = (nc.vector if j % 2 == 0
                                           else nc.gpsimd)
                                    eng.tensor_add(
                                        acc[:, off:], acc[:, off:],
                                        tgt[:, off:])
                                mm(psy[:, off:],
                                   v_sb[:, j, h * P:(h + 1) * P],
                                   tgt[:, off:],
                                   start=(j == 0), stop=(j == njt - 1))
                            den = psBC.tile([P, NT], F32, tag="den", bufs=1,
                                            name="den")
                            mm(den[:], ones_t[:], ps_e[:],
                               start=True, stop=False)
                            mm(den[:], ones_t[:], ps_o[:],
                               start=False, stop=True)
                            rec = wk.tile([P, NT], F32, tag="rec", bufs=1,
                                          name="rec")
                            nc.vector.reciprocal_approx_fast(rec[:], den[:])
                            nc.vector.tensor_mul(
                                y_t[h][:, t * NT:(t + 1) * NT], psy[:],
                                rec[:])

                    for h in range(HPC):
                        emit_B(2 * h)
                        emit_B(2 * h + 1)
                        emit_C(h)

            # ---------------- phase D: projection ----------------
            with (
                tc.tile_pool(name="wp_p", bufs=1) as wp_p,
                tc.tile_pool(name="ob_p", bufs=1) as ob_p,
                tc.tile_pool(name="psD", bufs=1, space="PSUM") as psD_p,
            ):
                wp_t = []
                for hh in range(HPC):
                    w_ = wp_p.tile([P, C], F16, tag=f"wp{hh}", name=f"wp{hh}")
                    nc.gpsimd.dma_start(w_[:], wp[hh * P:(hh + 1) * P, :])
                    wp_t.append(w_)
                for m in range(T // P):
                    msl = slice(m * P, (m + 1) * P)
                    psD = [
                        psD_p.tile([P, NT], F32, tag="psD", bufs=8,
                                   name="psD")
                        for _ in range(4)
                    ]
                    for hh in range(HPC):
                        lhsT = y_t[hh][:, msl]
                        for oc in range(4):
                            mm(psD[oc][:], lhsT,
                               wp_t[hh][:, oc * NT:(oc + 1) * NT],
                               start=(hh == 0), stop=(hh == HPC - 1))
                    ob = ob_p.tile([P, C], F32, tag="ob", bufs=3, name="ob")
                    for oc in range(4):
                        dst = ob[:, oc * NT:(oc + 1) * NT]
                        if oc % 2 == 0:
                            nc.scalar.copy(dst, psD[oc][:])
                        else:
                            nc.vector.tensor_copy(dst, psD[oc][:])
                    nc.sync.dma_start(out[msl, :], ob[:])

    nc.finalize()
    return nc


def _chunked(a, inner):
    """[CC*P, inner_cols] -> [P, CC*inner_cols] with (p, c, n) layout."""
    return np.ascontiguousarray(
        a.reshape(CC, P, inner).transpose(1, 0, 2).reshape(P, CC * inner))


def prep_inputs(x, w_attn, b_attn, w_proj, b_proj):
    """Build the 8 per-core input maps from full inputs."""
    x = np.asarray(x, dtype=np.float32)
    w_attn = np.asarray(w_attn, dtype=np.float32)
    b_attn = np.asarray(b_attn, dtype=np.float32)
    w_proj = np.asarray(w_proj, dtype=np.float32)

    scale = np.float32(1.0 / np.sqrt(D))

    inv_freq = 1.0 / (ROPE_BASE ** (np.arange(0, D, 2, dtype=np.float32) / D))
    tpos = np.arange(T, dtype=np.float32)
    ang = np.outer(tpos, inv_freq)  # [T, 64]
    cos_t, sin_t = np.cos(ang).T, np.sin(ang).T  # [64, T]
    # 1/64-scaled: the QKV psums carry a 64x weight scale; RoPE undoes it
    cs = np.ascontiguousarray(
        (np.concatenate([cos_t, cos_t], axis=0) / 64.0).astype(np.float16))
    sw = np.ascontiguousarray(
        (np.concatenate([-sin_t, sin_t], axis=0) / 64.0).astype(np.float16))

    qq = np.arange(P)
    kk = np.arange(P)[:, None]
    tri = np.ascontiguousarray(
        (qq[None, :] >= kk).astype(np.float16))  # [128,128] causal triangle
    onesm = np.ones((P, P), dtype=np.float16)

    in_maps = []
    for core in range(8):
        b = core // 2
        hg = core % 2
        heads = list(range(hg * HPC, (hg + 1) * HPC))

        # interleaved feature order: q_0, k_0, q_1, k_1, ...
        wqk_rows = []
        bqk_cols = []
        for h in heads:
            qc = np.arange(h * D, (h + 1) * D)
            kc = qc + C
            for cols, s in ((qc, scale), (kc, np.float32(1.0))):
                wsel = w_attn[:, cols] * s  # [2048, 128]
                wqk_rows.append(_chunked(wsel, P))  # [128, 2048]
                bqk_cols.append(
                    (b_attn[cols] * s * 64.0).astype(np.float32))
        # weights std ~0.022 sits in e4m3's subnormal range; pre-scale by
        # 64 (descaled via ACT scale=1/64 on the PSUM read) so fp8 gets a
        # full mantissa. Patch fp16 weights get the same scale so the
        # shared PSUM banks stay uniform.
        wqk_f32 = np.concatenate(wqk_rows, axis=0) * 64.0  # [16*128, 2048]
        wqk8_s = np.ascontiguousarray(wqk_f32.astype(NP8))
        wqk16_s = np.ascontiguousarray(wqk_f32.astype(np.float16))
        bqk_s = np.ascontiguousarray(np.stack(bqk_cols, axis=1))  # [128, 16]

        qcols = np.concatenate(
            [np.arange(h * D, (h + 1) * D) for h in heads])
        vcols = qcols + 2 * C
        wv_f32 = w_attn[:, vcols] * 64.0  # [2048, 1024]
        wv16_s = np.ascontiguousarray(wv_f32.astype(np.float16))
        wv8_s = _chunked(wv_f32, HPC * D).astype(NP8)  # [128, 16*1024]
        wp_s = np.ascontiguousarray(w_proj[qcols, :].astype(np.float16))

        xT = x[b].T  # [C, T]
        xt16p_s = _chunked(xT[:, 0:PATCH], PATCH).astype(np.float16)
        # xt8: per t, [128, 16*512] chunked; stacked rows [4*128, 16*512]
        xt8_s = np.concatenate(
            [_chunked(xT[:, t * NT:(t + 1) * NT], NT) for t in range(TT)],
            axis=0).astype(NP8)

        in_maps.append({
            "xt8": np.ascontiguousarray(xt8_s),
            "wv8": np.ascontiguousarray(wv8_s),
            "wqk8": wqk8_s,
            "xt16p": np.ascontiguousarray(xt16p_s),
            "wv16": wv16_s,
            "wqk16": wqk16_s,
            "bqk": bqk_s,
            "cs": cs, "sw": sw, "tri": tri, "onesm": onesm, "wp": wp_s,
        })
    return in_maps


def _get_program():
    if "nc" not in _CACHE:
        _CACHE["nc"] = build_program()
    return _CACHE["nc"]


def _postprocess(outs, b_attn, w_proj, b_proj):
    b_attn = np.asarray(b_attn, dtype=np.float32)
    w_proj = np.asarray(w_proj, dtype=np.float32)
    b_proj = np.asarray(b_proj, dtype=np.float32)
    # v-bias and proj-bias are linear terms folded in on the host
    bias_full = b_attn[2 * C:3 * C] @ w_proj + b_proj  # [C]
    return np.stack(
        [outs[2 * b] + outs[2 * b + 1] + bias_full[None, :]
         for b in range(B)]
    ).astype(np.float32)


def _run(inputs, trace=False):
    from concourse.bass_utils import run_bass_kernel_spmd

    nc = _get_program()
    in_maps = prep_inputs(
        inputs["x"], inputs["w_attn"], inputs["b_attn"],
        inputs["w_proj"], inputs["b_proj"],
    )
    res = run_bass_kernel_spmd(nc, in_maps, core_ids=list(range(8)),
                               trace=trace)
    full = _postprocess([r["out"] for r in res.results],
                        inputs["b_attn"], inputs["w_proj"],
                        inputs["b_proj"])
    return full, res


def kernel(**inputs):
    full, _ = _run(inputs, trace=False)
    return full


if __name__ == "__main__":
    _get_program()
    print("built ok")
